# revision 4
# baseline (speedup 1.0000x reference)
"""TRN2 Bass kernel for nn_Attention (B=2, S=2048, DIM=2048, 16 heads).

Sharding: tensor-parallel over heads — 8 cores x 2 heads each.
Each core computes q/k/v projections for its 2 heads over both batches,
causal attention, and a partial output projection (row-parallel wo).
Host sums the 8 partial outputs.

Layouts (per core):
  xS   [16, 128, 16, 256]  = x.T chunked contiguous per s-chunk (replicated)
  wqT  [2048(k), 256(dq)]  = wq[head rows].T                  (sharded)
  wkT, wvT likewise; woT [256(dc), 2048(m)] = wo[:, head cols].T
  outp [2048(m), 4096(s)]  partial of out.T                   (summed on host)

All matmul operands are bf16 (PSUM accumulation stays fp32). The softmax
denominator is accumulated on the Vector engine (exacc) and reduced across
partitions on GpSimd (partition_all_reduce), keeping the row-sum work off
the Tensor engine.
"""

import sys

sys.path.insert(0, "/opt/trn_rl_repo")

import numpy as np

DIM = 2048
HEADS = 16
HD = 128
B = 2
S = 2048
SG = B * S  # 4096 global sequence (batch-major)
NCORES = 8
HPC = HEADS // NCORES  # 2 heads per core
DPC = HPC * HD  # 256 dims per core
KC = DIM // 128  # 16 contraction chunks
PC = 256  # projection s-chunk width
NPC = S // PC  # 8 proj chunks per batch
AC = 512  # attention sq-chunk width
NAC = S // AC  # 4 attention chunks per batch
ISQ = 1.0 / np.sqrt(np.float32(HD))

# softmax denominator reduction: "gpsimd" = partition_all_reduce on GpSimd,
# "pe" = ones-matmul + broadcast matmul on the Tensor engine (fallback).
_ALLREDUCE_MODE = "gpsimd"

_prog_cache = {}


def _build_program():
    import concourse.bass as bass
    from concourse import bacc
    from concourse import bass_isa
    import concourse.mybir as mybir
    import concourse.tile as tile

    # Route Exp AND Ln to the one table set containing both, so the ACT
    # table is loaded once instead of thrashing between per-function sets
    # (~1.3us per reload, 2 reloads per softmax-normalize otherwise).
    if not getattr(bacc, "_act_tables_patched", False):
        _orig_get_tables = bacc.get_activation_tables
        _E = mybir.ActivationFunctionType.Exp
        _L = mybir.ActivationFunctionType.Ln

        def _patched_get_tables(arch):
            tabs = dict(_orig_get_tables(arch))
            both = {
                n for n, fns in tabs.items() if _E in fns and _L in fns
            }
            if both:
                keep = sorted(both)[0]
                tabs = {
                    n: (fns if n == keep else fns - {_E, _L})
                    for n, fns in tabs.items()
                }
            return tabs

        bacc.get_activation_tables = _patched_get_tables
        bacc._act_tables_patched = True

    f32 = mybir.dt.float32
    bf = mybir.dt.bfloat16
    EXP = mybir.ActivationFunctionType.Exp
    LOG = mybir.ActivationFunctionType.Ln

    nc = bacc.Bacc()

    xS = nc.dram_tensor("xS", [SG // PC, 128, KC, PC], bf, kind="ExternalInput")
    wqT = nc.dram_tensor("wqT", [DIM, DPC], bf, kind="ExternalInput")
    wkT = nc.dram_tensor("wkT", [DIM, DPC], bf, kind="ExternalInput")
    wvT = nc.dram_tensor("wvT", [DIM, DPC], bf, kind="ExternalInput")
    woT = nc.dram_tensor("woT", [DPC, DIM], bf, kind="ExternalInput")
    m01x = nc.dram_tensor("m01x", [128, 1024], bf, kind="ExternalInput")
    onesA = nc.dram_tensor("onesA", [128, 1], bf, kind="ExternalInput")
    onesB = nc.dram_tensor("onesB", [1, 128], bf, kind="ExternalInput")
    outp = nc.dram_tensor("outp", [DIM, SG], bf, kind="ExternalOutput")

    with tile.TileContext(nc) as tc:
        with (
            tc.tile_pool(name="wpool", bufs=1) as wpool,
            tc.tile_pool(name="xpool", bufs=3) as xpool,
            tc.tile_pool(name="kv", bufs=1) as kvpool,
            tc.tile_pool(name="work", bufs=2) as work,
            tc.tile_pool(name="expool", bufs=3) as expool,
            tc.tile_pool(name="ps", bufs=1, space="PSUM") as ps,
        ):
            # --- resident constants / weights ---
            wqr = wpool.tile([128, KC, DPC], bf, tag="wqr")
            wkr = wpool.tile([128, KC, DPC], bf, tag="wkr")
            wvr = wpool.tile([128, KC, DPC], bf, tag="wvr")
            wor = wpool.tile([128, HPC, DIM], bf, tag="wor")
            m01 = wpool.tile([128, 1024], bf, tag="m01")
            onA = wpool.tile([128, 1], bf, tag="onA")
            onB = wpool.tile([1, 128], bf, tag="onB")

            def emit_w_dmas(wtile, wdram):
                for kc in range(KC):
                    ksl = slice(kc * 128, (kc + 1) * 128)
                    nc.sync.dma_start(wtile[:, kc, :], wdram[ksl, :])

            def emit_late_dmas():
                for dc in range(HPC):
                    nc.sync.dma_start(
                        wor[:, dc, :], woT[dc * 128 : (dc + 1) * 128, :]
                    )
                nc.sync.dma_start(onA[:], onesA[:])
                nc.sync.dma_start(onB[:], onesB[:])

            # resident per-core activations
            kTr = kvpool.tile([128, B * HPC, S], bf, tag="kTr")  # [d, bh, s]
            vr = kvpool.tile([128, B * (S // 128), DPC], bf, tag="vr")  # [s%, blk, d]

            def proj_units(b, j, qTc):
                dmas = []
                units = []
                for half in range(AC // PC):
                    cl = (AC // PC) * j + half
                    xa = xpool.tile(
                        [128, KC, PC], bf, tag="xa", name=f"xa_{b}_{j}_{half}"
                    )

                    cg = b * NPC + cl

                    def dma_unit(xa=xa, cg=cg):
                        for qt in range(4):
                            nc.sync.dma_start(
                                xa[:, qt * 4 : (qt + 1) * 4, :],
                                xS[cg, :, qt * 4 : (qt + 1) * 4, :],
                            )

                    dmas.append(dma_unit)
                    for h in range(HPC):
                        def q_unit(h=h, xa=xa, half=half):
                            dsl = slice(h * 128, (h + 1) * 128)
                            pq = ps.tile([128, PC], f32, tag="pq", bufs=2)
                            for kc in range(KC):
                                nc.tensor.matmul(
                                    pq[:], wqr[:, kc, dsl], xa[:, kc, :],
                                    start=(kc == 0), stop=(kc == KC - 1),
                                )
                            nc.vector.tensor_copy(
                                qTc[:, h, half * PC : (half + 1) * PC], pq[:]
                            )

                        def k_unit(h=h, xa=xa, cl=cl):
                            dsl = slice(h * 128, (h + 1) * 128)
                            pk = ps.tile([128, PC], f32, tag="pq", bufs=2)
                            for kc in range(KC):
                                nc.tensor.matmul(
                                    pk[:], wkr[:, kc, dsl], xa[:, kc, :],
                                    start=(kc == 0), stop=(kc == KC - 1),
                                )
                            nc.vector.tensor_copy(
                                kTr[:, b * HPC + h, cl * PC : (cl + 1) * PC], pk[:]
                            )

                        units.append(q_unit)
                        units.append(k_unit)
                    for sb in range(PC // 128):
                        def v_unit(sb=sb, xa=xa, cl=cl):
                            pv = ps.tile([128, DPC], f32, tag="pq", bufs=2)
                            for kc in range(KC):
                                nc.tensor.matmul(
                                    pv[:], xa[:, kc, sb * 128 : (sb + 1) * 128],
                                    wvr[:, kc, :],
                                    start=(kc == 0), stop=(kc == KC - 1),
                                )
                            vblk = b * (S // 128) + cl * (PC // 128) + sb
                            nc.vector.tensor_copy(vr[:, vblk, :], pv[:])

                        units.append(v_unit)
                return dmas + units

            def att_units(b, j, qTc, uS):
                units = []
                for h in range(HPC):
                    bh = b * HPC + h
                    nblocks = (j + 1) * (AC // 128)
                    nfull = j * (AC // 128)
                    box = {}

                    def head_start(box=box, h=h):
                        box["U"] = ps.tile([128, AC], f32, tag="u", bufs=2,
                                           name=f"U_{b}_{j}_{h}")
                        box["exacc"] = work.tile(
                            [128, AC], f32, tag="exacc", bufs=2,
                            name=f"exacc_{b}_{j}_{h}"
                        )
                        if _ALLREDUCE_MODE == "pe":
                            box["se"] = ps.tile([1, AC], f32, tag="se", bufs=1,
                                                name=f"se_{b}_{j}_{h}")

                    for i in range(nblocks):
                        def block_unit(i=i, h=h, bh=bh, box=box,
                                       nblocks=nblocks, nfull=nfull):
                            if i == 0:
                                head_start(box, h)
                            U = box["U"]
                            exacc = box["exacc"]
                            loc = max(0, 128 * i - AC * j)
                            sc = ps.tile([128, AC], f32, tag="sc", bufs=3)
                            ex = expool.tile([128, AC], bf, tag="ex", bufs=5)
                            nc.tensor.matmul(
                                sc[:, loc:AC],
                                kTr[:, bh, i * 128 : (i + 1) * 128],
                                qTc[:, h, loc:AC],
                                start=True, stop=True,
                            )
                            if i < nfull:
                                nc.scalar.activation(ex[:], sc[:], EXP, scale=ISQ)
                            else:
                                ds = expool.tile([128, AC], bf, tag="ds", bufs=2)
                                nc.scalar.activation(
                                    ds[:, loc:AC], sc[:, loc:AC], EXP, scale=ISQ
                                )
                                nc.vector.tensor_mul(
                                    ex[:, loc:AC], ds[:, loc:AC],
                                    m01[:, 384 : 384 + AC - loc],
                                )
                            if i == 0:
                                nc.vector.tensor_copy(exacc[:], ex[:])
                            else:
                                nc.vector.tensor_add(
                                    exacc[:, loc:AC], exacc[:, loc:AC],
                                    ex[:, loc:AC],
                                )
                            vblk = b * (S // 128) + i
                            nc.tensor.matmul(
                                U[:, loc:AC],
                                vr[:, vblk, h * 128 : (h + 1) * 128],
                                ex[:, loc:AC],
                                start=(i == 0), stop=(i == nblocks - 1),
                            )
                            if _ALLREDUCE_MODE == "pe":
                                nc.tensor.matmul(
                                    box["se"][:, loc:AC], onA[:], ex[:, loc:AC],
                                    start=(i == 0), stop=(i == nblocks - 1),
                                )

                        units.append(block_unit)

                    if _ALLREDUCE_MODE == "gpsimd":
                        def red_unit(h=h, box=box):
                            seb = work.tile(
                                [128, AC], f32, tag="seb", name=f"seb_{b}_{j}_{h}"
                            )
                            nc.gpsimd.partition_all_reduce(
                                seb[:], box["exacc"][:], 128,
                                bass_isa.ReduceOp.add,
                            )
                            box["seb"] = seb

                        def fin_unit(h=h, box=box):
                            tb = work.tile([128, AC], f32, tag="tb",
                                           name=f"tb_{b}_{j}_{h}")
                            nc.scalar.activation(tb[:], box["seb"][:], LOG)
                            rb = work.tile([128, AC], f32, tag="rb")
                            nc.scalar.activation(rb[:], tb[:], EXP, scale=-1.0)
                            nc.vector.tensor_mul(uS[:, h, :], box["U"][:], rb[:])

                        units.append(red_unit)
                        units.append(fin_unit)
                    else:
                        def ln_unit(h=h, box=box):
                            lnz = work.tile([1, AC], f32, tag="lnz",
                                            name=f"lnz_{b}_{j}_{h}")
                            nc.scalar.activation(lnz[:], box["se"][:], LOG)
                            box["lnz"] = lnz

                        def fin_unit(h=h, box=box):
                            bc = ps.tile([128, AC], f32, tag="sc", bufs=3)
                            nc.tensor.matmul(
                                bc[:], onB[:], box["lnz"][:], start=True, stop=True
                            )
                            rb = work.tile([128, AC], f32, tag="rb")
                            nc.scalar.activation(rb[:], bc[:], EXP, scale=-1.0)
                            nc.vector.tensor_mul(uS[:, h, :], box["U"][:], rb[:])

                        units.append(ln_unit)
                        units.append(fin_unit)
                return units

            def out_units(b, j, uS, tags=("po",)):
                units = []
                sg0 = b * S + j * AC
                for mb in range(DIM // 128):
                    def o_unit(mb=mb):
                        tg = tags[mb % len(tags)]
                        po = ps.tile(
                            [128, AC], f32, tag=tg, bufs=(1 if tg == "po" else 2)
                        )
                        for dc in range(HPC):
                            nc.tensor.matmul(
                                po[:],
                                wor[:, dc, mb * 128 : (mb + 1) * 128],
                                uS[:, dc, :],
                                start=(dc == 0), stop=(dc == HPC - 1),
                            )
                        ob = work.tile([128, AC], bf, tag="ob")
                        if mb % 3 == 2:
                            nc.scalar.copy(ob[:], po[:])
                        else:
                            nc.vector.tensor_copy(ob[:], po[:])
                        nc.sync.dma_start(
                            outp[mb * 128 : (mb + 1) * 128, sg0 : sg0 + AC], ob[:]
                        )

                    units.append(o_unit)
                return units

            def merge_emit(a_units, b_units):
                na, nb = len(a_units), len(b_units)
                ia = ib = 0
                while ia < na or ib < nb:
                    fa = ia / na if na else 2.0
                    fb = ib / nb if nb else 2.0
                    if fa <= fb:
                        a_units[ia]()
                        ia += 1
                    else:
                        b_units[ib]()
                        ib += 1

            # software pipeline: att(c) interleaved with proj(c+1) + out(c-1)
            chunks = [(b, j) for b in range(B) for j in range(NAC)]
            qTcs = {}
            uSs = {}
            qTcs[chunks[0]] = work.tile([128, HPC, AC], bf, tag="qTc", name="qTc0")
            u0 = proj_units(*chunks[0], qTcs[chunks[0]])
            # startup order: chunk-0 x DMAs, then weights in first-use order
            # (wq for the q matmuls, wk, wv+mask), then chunk-0 compute; the
            # wo/ones DMAs are emitted only after the first chunk's work.
            u0[0]()
            u0[1]()
            emit_w_dmas(wqr, wqT)
            emit_w_dmas(wkr, wkT)
            emit_w_dmas(wvr, wvT)
            nc.sync.dma_start(m01[:], m01x[:])
            for u in u0[2:]:
                u()
            emit_late_dmas()
            for idx, (b, j) in enumerate(chunks):
                fill = []
                if idx + 1 < len(chunks):
                    nb_, nj_ = chunks[idx + 1]
                    qTcs[(nb_, nj_)] = work.tile(
                        [128, HPC, AC], bf, tag="qTc", name=f"qTc_{nb_}_{nj_}"
                    )
                    fill += proj_units(nb_, nj_, qTcs[(nb_, nj_)])
                if idx > 0:
                    fill += out_units(*chunks[idx - 1], uSs.pop(chunks[idx - 1]))
                uS = work.tile([128, HPC, AC], bf, tag="uS", name=f"uS_{b}_{j}")
                uSs[(b, j)] = uS
                merge_emit(att_units(b, j, qTcs.pop((b, j)), uS), fill)
            for u in out_units(*chunks[-1], uSs.pop(chunks[-1]), tags=("po", "u")):
                u()

    nc.finalize()
    return nc


def _get_program():
    key = "prog"
    if key not in _prog_cache:
        _prog_cache[key] = _build_program()
    return _prog_cache[key]


def _is_causal_neg_mask(mask):
    m = mask.reshape(S, S)
    tri = np.triu(np.ones((S, S), dtype=bool), k=1)
    return (
        np.all(m[~tri] == 0.0)
        and np.all(m[tri] <= -1e8)
        and np.all(np.isfinite(m) | tri)
    )


def _reference_fallback(x, mask, wq, wk, wv, wo):
    xf = x.astype(np.float32)
    q = (xf @ wq.T).reshape(B, S, HEADS, HD).transpose(0, 2, 1, 3)
    k = (xf @ wk.T).reshape(B, S, HEADS, HD).transpose(0, 2, 1, 3)
    v = (xf @ wv.T).reshape(B, S, HEADS, HD).transpose(0, 2, 1, 3)
    scores = np.matmul(q, k.transpose(0, 1, 3, 2)) / np.sqrt(np.float32(HD))
    scores = scores + mask
    scores = scores - scores.max(axis=-1, keepdims=True)
    e = np.exp(scores)
    probs = e / e.sum(axis=-1, keepdims=True)
    out = np.matmul(probs, v)
    out = out.transpose(0, 2, 1, 3).reshape(B, S, HEADS * HD)
    return (out @ wo.T).astype(np.float32)


def kernel(x, mask, wq, wk, wv, wo):
    import ml_dtypes

    bf16 = ml_dtypes.bfloat16

    x = np.ascontiguousarray(np.asarray(x, dtype=np.float32))
    mask = np.asarray(mask, dtype=np.float32)
    wq = np.ascontiguousarray(np.asarray(wq, dtype=np.float32))
    wk = np.ascontiguousarray(np.asarray(wk, dtype=np.float32))
    wv = np.ascontiguousarray(np.asarray(wv, dtype=np.float32))
    wo = np.ascontiguousarray(np.asarray(wo, dtype=np.float32))

    if not _is_causal_neg_mask(mask):
        return _reference_fallback(x, mask, wq, wk, wv, wo)

    from concourse.bass_utils import run_bass_kernel_spmd

    nc = _get_program()

    xT = x.reshape(SG, DIM).T  # [DIM, SG]
    # xS[cg, p, kc, s'] = xT[kc*128+p, cg*PC+s'] (contiguous per chunk)
    xS = xT.reshape(KC, 128, SG // PC, PC).transpose(2, 1, 0, 3).astype(
        bf16, order="C"
    )
    # m01big[k, c] = 1.0 iff (c - 384) >= k; partial blocks slice [384:384+N)
    kk = np.arange(128)[:, None]
    cc = np.arange(1024)[None, :]
    m01x = ((cc - 384) >= kk).astype(bf16)
    onesA = np.ones((128, 1), dtype=bf16)
    onesB = np.ones((1, 128), dtype=bf16)

    in_maps = []
    for c in range(NCORES):
        hs = slice(c * DPC, (c + 1) * DPC)
        in_maps.append(
            {
                "xS": xS,
                "wqT": wq[hs, :].T.astype(bf16, order="C"),
                "wkT": wk[hs, :].T.astype(bf16, order="C"),
                "wvT": wv[hs, :].T.astype(bf16, order="C"),
                "woT": wo[:, hs].T.astype(bf16, order="C"),
                "m01x": m01x,
                "onesA": onesA,
                "onesB": onesB,
            }
        )

    global LAST_RESULT
    for attempt in range(3):
        res = run_bass_kernel_spmd(nc, in_maps, list(range(NCORES)))
        LAST_RESULT = res
        acc = np.asarray(res.results[0]["outp"]).astype(np.float32)
        for c in range(1, NCORES):
            acc += np.asarray(res.results[c]["outp"]).astype(np.float32)
        # guard against rare transient device glitches (non-finite output)
        if np.isfinite(acc).all():
            break
    # outp is out.T: [m, s_glob] -> [B, S, DIM]
    return np.ascontiguousarray(acc.T).reshape(B, S, DIM)


if __name__ == "__main__":
    rng = np.random.default_rng(0)
    x = rng.standard_normal((B, S, DIM), dtype=np.float32)
    neg = np.float32(-1e9)
    maskm = np.triu(np.full((S, S), neg, dtype=np.float32), k=1)[None, None]
    ws = [rng.standard_normal((DIM, DIM), dtype=np.float32) * 0.02 for _ in range(4)]
    out = kernel(x, maskm, *ws)
    print(out.shape, out.dtype)


# revision 8
# speedup vs baseline: 1.0759x; 1.0759x over previous
"""TRN2 Bass kernel for nn_Attention (B=2, S=2048, DIM=2048, 16 heads).

Sharding: tensor-parallel over heads — 8 cores x 2 heads each.
Each core computes q/k/v projections for its 2 heads over both batches,
causal attention, and a partial output projection (row-parallel wo).
Host sums the 8 partial outputs.

Layouts (per core):
  xS   [8, 128, 16, 512]   = x.T chunked contiguous per s-chunk (replicated)
  wqT  [2048(k), 256(dq)]  = wq[head rows].T                  (sharded)
  wkT, wvT likewise; woT [256(dc), 2048(m)] = wo[:, head cols].T
  outp [2048(m), 4096(s)]  partial of out.T (bf16, summed on host)

All matmul operands are bf16 (PSUM accumulation stays fp32), except the
softmax-denominator path which runs in float32r. The denominator is
accumulated per key-block into exacc tiles (split across Vector and GpSimd
engines), then folded across partitions with a single ones-matmul per
(batch, chunk, head) instead of one per key-block.
"""

import sys

sys.path.insert(0, "/opt/trn_rl_repo")

import numpy as np

DIM = 2048
HEADS = 16
HD = 128
B = 2
S = 2048
SG = B * S  # 4096 global sequence (batch-major)
NCORES = 8
HPC = HEADS // NCORES  # 2 heads per core
DPC = HPC * HD  # 256 dims per core
KC = DIM // 128  # 16 contraction chunks
AC = 512  # chunk width (projection and attention)
NAC = S // AC  # 4 chunks per batch
ISQ = 1.0 / np.sqrt(np.float32(HD))

_prog_cache = {}


def _build_program():
    import concourse.bass as bass
    from concourse import bacc
    import concourse.mybir as mybir
    import concourse.tile as tile

    # Route Exp AND Ln to the one table set containing both, so the ACT
    # table is loaded once instead of thrashing between per-function sets
    # (~1.3us per reload, 2 reloads per softmax-normalize otherwise).
    if not getattr(bacc, "_act_tables_patched", False):
        _orig_get_tables = bacc.get_activation_tables
        _E = mybir.ActivationFunctionType.Exp
        _L = mybir.ActivationFunctionType.Ln

        def _patched_get_tables(arch):
            tabs = dict(_orig_get_tables(arch))
            both = {
                n for n, fns in tabs.items() if _E in fns and _L in fns
            }
            if both:
                keep = sorted(both)[0]
                tabs = {
                    n: (fns if n == keep else fns - {_E, _L})
                    for n, fns in tabs.items()
                }
            return tabs

        bacc.get_activation_tables = _patched_get_tables
        bacc._act_tables_patched = True

    f32 = mybir.dt.float32
    fr = mybir.dt.float32r
    bf = mybir.dt.bfloat16
    EXP = mybir.ActivationFunctionType.Exp
    LOG = mybir.ActivationFunctionType.Ln

    nc = bacc.Bacc()

    xS = nc.dram_tensor("xS", [SG // AC, 128, KC, AC], bf, kind="ExternalInput")
    wqT = nc.dram_tensor("wqT", [DIM, DPC], bf, kind="ExternalInput")
    wkT = nc.dram_tensor("wkT", [DIM, DPC], bf, kind="ExternalInput")
    wvT = nc.dram_tensor("wvT", [DIM, DPC], bf, kind="ExternalInput")
    woT = nc.dram_tensor("woT", [DPC, DIM], bf, kind="ExternalInput")
    m01x = nc.dram_tensor("m01x", [128, 1024], bf, kind="ExternalInput")
    onesA = nc.dram_tensor("onesA", [128, 1], fr, kind="ExternalInput")
    onesB = nc.dram_tensor("onesB", [1, 128], fr, kind="ExternalInput")
    outp = nc.dram_tensor("outp", [DIM, SG], bf, kind="ExternalOutput")

    with tile.TileContext(nc) as tc:
        with (
            tc.tile_pool(name="wpool", bufs=1) as wpool,
            tc.tile_pool(name="xpool", bufs=3) as xpool,
            tc.tile_pool(name="kv", bufs=1) as kvpool,
            tc.tile_pool(name="work", bufs=2) as work,
            tc.tile_pool(name="expool", bufs=3) as expool,
            tc.tile_pool(name="ps", bufs=1, space="PSUM") as ps,
        ):
            # --- resident constants / weights ---
            wqr = wpool.tile([128, KC, DPC], bf, tag="wqr")
            wkr = wpool.tile([128, KC, DPC], bf, tag="wkr")
            wvr = wpool.tile([128, KC, DPC], bf, tag="wvr")
            wor = wpool.tile([128, HPC, DIM], bf, tag="wor")
            m01 = wpool.tile([128, 1024], bf, tag="m01")
            onA = wpool.tile([128, 1], fr, tag="onA")
            onB = wpool.tile([1, 128], fr, tag="onB")

            def emit_w_dmas(wtile, wdram):
                for kc in range(KC):
                    ksl = slice(kc * 128, (kc + 1) * 128)
                    nc.sync.dma_start(wtile[:, kc, :], wdram[ksl, :])

            def emit_late_dmas():
                for dc in range(HPC):
                    nc.sync.dma_start(
                        wor[:, dc, :], woT[dc * 128 : (dc + 1) * 128, :]
                    )
                nc.sync.dma_start(onA[:], onesA[:])
                nc.sync.dma_start(onB[:], onesB[:])

            # resident per-core activations
            kTr = kvpool.tile([128, B * HPC, S], bf, tag="kTr")  # [d, bh, s]
            vr = kvpool.tile([128, B * (S // 128), DPC], bf, tag="vr")  # [s%, blk, d]

            def proj_units(b, j, qTc):
                xa = xpool.tile([128, KC, AC], bf, tag="xa", name=f"xa_{b}_{j}")
                cg = b * NAC + j

                def dma_unit(xa=xa, cg=cg):
                    for qt in range(8):
                        nc.sync.dma_start(
                            xa[:, qt * 2 : (qt + 1) * 2, :],
                            xS[cg, :, qt * 2 : (qt + 1) * 2, :],
                        )

                def q_unit(h, xa=xa):
                    dsl = slice(h * 128, (h + 1) * 128)
                    pq = ps.tile([128, AC], f32, tag="pq", bufs=2)
                    for kc in range(KC):
                        nc.tensor.matmul(
                            pq[:], wqr[:, kc, dsl], xa[:, kc, :],
                            start=(kc == 0), stop=(kc == KC - 1),
                        )
                    nc.vector.tensor_copy(qTc[:, h, :], pq[:])

                def k_unit(h, xa=xa):
                    dsl = slice(h * 128, (h + 1) * 128)
                    pk = ps.tile([128, AC], f32, tag="pq", bufs=2)
                    for kc in range(KC):
                        nc.tensor.matmul(
                            pk[:], wkr[:, kc, dsl], xa[:, kc, :],
                            start=(kc == 0), stop=(kc == KC - 1),
                        )
                    nc.vector.tensor_copy(
                        kTr[:, b * HPC + h, j * AC : (j + 1) * AC], pk[:]
                    )

                def v_unit(sb, xa=xa):
                    pv = ps.tile([128, DPC], f32, tag="pq", bufs=2)
                    for kc in range(KC):
                        nc.tensor.matmul(
                            pv[:], xa[:, kc, sb * 128 : (sb + 1) * 128],
                            wvr[:, kc, :],
                            start=(kc == 0), stop=(kc == KC - 1),
                        )
                    vblk = b * (S // 128) + j * (AC // 128) + sb
                    nc.vector.tensor_copy(vr[:, vblk, :], pv[:])

                units = [
                    lambda: q_unit(0), lambda: k_unit(0),
                    lambda: v_unit(0), lambda: v_unit(1),
                    lambda: q_unit(1), lambda: k_unit(1),
                    lambda: v_unit(2), lambda: v_unit(3),
                ]
                return [dma_unit] + units

            def att_units(b, j, qTc, uS):
                units = []
                for h in range(HPC):
                    bh = b * HPC + h
                    nblocks = (j + 1) * (AC // 128)
                    nfull = j * (AC // 128)
                    box = {}

                    def head_start(box=box, h=h):
                        box["U"] = ps.tile([128, AC], f32, tag="u", bufs=2,
                                           name=f"U_{b}_{j}_{h}")
                        # two denominator accumulators: even blocks on DVE,
                        # odd blocks on GpSimd — halves the serial add chain
                        box["eA"] = work.tile([128, AC], fr, tag="eA", bufs=2,
                                              name=f"eA_{b}_{j}_{h}")
                        box["eB"] = work.tile([128, AC], fr, tag="eB", bufs=2,
                                              name=f"eB_{b}_{j}_{h}")

                    for i in range(nblocks):
                        def block_unit(i=i, h=h, bh=bh, box=box,
                                       nblocks=nblocks, nfull=nfull):
                            if i == 0:
                                head_start(box, h)
                            U = box["U"]
                            loc = max(0, 128 * i - AC * j)
                            sc = ps.tile([128, AC], f32, tag="sc", bufs=2)
                            ex = expool.tile([128, AC], bf, tag="ex", bufs=5)
                            nc.tensor.matmul(
                                sc[:, loc:AC],
                                kTr[:, bh, i * 128 : (i + 1) * 128],
                                qTc[:, h, loc:AC],
                                start=True, stop=True,
                            )
                            if i < nfull:
                                nc.scalar.activation(ex[:], sc[:], EXP, scale=ISQ)
                            else:
                                ds = expool.tile([128, AC], bf, tag="ds", bufs=2)
                                nc.scalar.activation(
                                    ds[:, loc:AC], sc[:, loc:AC], EXP, scale=ISQ
                                )
                                nc.vector.tensor_mul(
                                    ex[:, loc:AC], ds[:, loc:AC],
                                    m01[:, 384 : 384 + AC - loc],
                                )
                            acc = box["eB"] if i % 2 == 0 else box["eA"]
                            if i < 2:
                                # block 0 always covers [0:AC); block 1 may be
                                # a diagonal block that only wrote [loc:AC) —
                                # record the offset so the se fold only reads
                                # the written range of eA.
                                if i == 1:
                                    box["l1"] = loc
                                nc.vector.tensor_copy(
                                    acc[:, loc:AC], ex[:, loc:AC]
                                )
                            elif i % 2 == 0:
                                nc.vector.tensor_add(
                                    acc[:, loc:AC], acc[:, loc:AC], ex[:, loc:AC]
                                )
                            else:
                                nc.gpsimd.tensor_add(
                                    acc[:, loc:AC], acc[:, loc:AC], ex[:, loc:AC]
                                )
                            vblk = b * (S // 128) + i
                            nc.tensor.matmul(
                                U[:, loc:AC],
                                vr[:, vblk, h * 128 : (h + 1) * 128],
                                ex[:, loc:AC],
                                start=(i == 0), stop=(i == nblocks - 1),
                            )

                        units.append(block_unit)

                    def red_unit(h=h, box=box):
                        l1 = box["l1"]
                        se = ps.tile([1, AC], f32, tag="se", bufs=1,
                                     name=f"se_{b}_{j}_{h}")
                        nc.tensor.matmul(
                            se[:], onA[:], box["eB"][:], start=True, stop=False
                        )
                        nc.tensor.matmul(
                            se[:, l1:AC], onA[:], box["eA"][:, l1:AC],
                            start=False, stop=True,
                        )
                        lnz = work.tile([1, AC], fr, tag="lnz",
                                        name=f"lnz_{b}_{j}_{h}")
                        nc.scalar.activation(lnz[:], se[:], LOG)
                        box["lnz"] = lnz

                    def fin_unit(h=h, box=box):
                        bc = ps.tile([128, AC], f32, tag="sc", bufs=2)
                        nc.tensor.matmul(
                            bc[:], onB[:], box["lnz"][:], start=True, stop=True
                        )
                        rb = work.tile([128, AC], f32, tag="rb")
                        nc.scalar.activation(rb[:], bc[:], EXP, scale=-1.0)
                        nc.vector.tensor_mul(uS[:, h, :], box["U"][:], rb[:])

                    units.append(red_unit)
                    units.append(fin_unit)
                return units

            def out_units(b, j, uS, tags=("po",)):
                units = []
                sg0 = b * S + j * AC
                for mb in range(DIM // 128):
                    def o_unit(mb=mb):
                        tg = tags[mb % len(tags)]
                        po = ps.tile(
                            [128, AC], f32, tag=tg, bufs=(1 if tg == "po" else 2)
                        )
                        for dc in range(HPC):
                            nc.tensor.matmul(
                                po[:],
                                wor[:, dc, mb * 128 : (mb + 1) * 128],
                                uS[:, dc, :],
                                start=(dc == 0), stop=(dc == HPC - 1),
                            )
                        ob = work.tile([128, AC], bf, tag="ob")
                        if mb % 3 == 2:
                            nc.scalar.copy(ob[:], po[:])
                        else:
                            nc.vector.tensor_copy(ob[:], po[:])
                        nc.sync.dma_start(
                            outp[mb * 128 : (mb + 1) * 128, sg0 : sg0 + AC], ob[:]
                        )

                    units.append(o_unit)
                return units

            def merge_emit(a_units, b_units):
                na, nb = len(a_units), len(b_units)
                ia = ib = 0
                while ia < na or ib < nb:
                    fa = ia / na if na else 2.0
                    fb = ib / nb if nb else 2.0
                    if fa <= fb:
                        a_units[ia]()
                        ia += 1
                    else:
                        b_units[ib]()
                        ib += 1

            # software pipeline: att(c) interleaved with proj(c+1) + out(c-1)
            chunks = [(b, j) for b in range(B) for j in range(NAC)]
            qTcs = {}
            uSs = {}
            qTcs[chunks[0]] = work.tile([128, HPC, AC], bf, tag="qTc", name="qTc0")
            u0 = proj_units(*chunks[0], qTcs[chunks[0]])
            # startup order: chunk-0 x DMAs, then weights in first-use order
            # (wq for the q matmuls, wk, wv+mask), then chunk-0 compute; the
            # wo/ones DMAs are emitted only after the first chunk's work.
            u0[0]()
            emit_w_dmas(wqr, wqT)
            emit_w_dmas(wkr, wkT)
            emit_w_dmas(wvr, wvT)
            nc.sync.dma_start(m01[:], m01x[:])
            for u in u0[1:]:
                u()
            emit_late_dmas()
            for idx, (b, j) in enumerate(chunks):
                fill = []
                if idx + 1 < len(chunks):
                    nb_, nj_ = chunks[idx + 1]
                    qTcs[(nb_, nj_)] = work.tile(
                        [128, HPC, AC], bf, tag="qTc", name=f"qTc_{nb_}_{nj_}"
                    )
                    fill += proj_units(nb_, nj_, qTcs[(nb_, nj_)])
                if idx > 0:
                    fill += out_units(*chunks[idx - 1], uSs.pop(chunks[idx - 1]))
                uS = work.tile([128, HPC, AC], bf, tag="uS", name=f"uS_{b}_{j}")
                uSs[(b, j)] = uS
                merge_emit(att_units(b, j, qTcs.pop((b, j)), uS), fill)
            for u in out_units(*chunks[-1], uSs.pop(chunks[-1]), tags=("po", "u")):
                u()

    nc.finalize()
    return nc


def _get_program():
    key = "prog"
    if key not in _prog_cache:
        _prog_cache[key] = _build_program()
    return _prog_cache[key]


def _is_causal_neg_mask(mask):
    m = mask.reshape(S, S)
    tri = np.triu(np.ones((S, S), dtype=bool), k=1)
    return (
        np.all(m[~tri] == 0.0)
        and np.all(m[tri] <= -1e8)
        and np.all(np.isfinite(m) | tri)
    )


def _reference_fallback(x, mask, wq, wk, wv, wo):
    xf = x.astype(np.float32)
    q = (xf @ wq.T).reshape(B, S, HEADS, HD).transpose(0, 2, 1, 3)
    k = (xf @ wk.T).reshape(B, S, HEADS, HD).transpose(0, 2, 1, 3)
    v = (xf @ wv.T).reshape(B, S, HEADS, HD).transpose(0, 2, 1, 3)
    scores = np.matmul(q, k.transpose(0, 1, 3, 2)) / np.sqrt(np.float32(HD))
    scores = scores + mask
    scores = scores - scores.max(axis=-1, keepdims=True)
    e = np.exp(scores)
    probs = e / e.sum(axis=-1, keepdims=True)
    out = np.matmul(probs, v)
    out = out.transpose(0, 2, 1, 3).reshape(B, S, HEADS * HD)
    return (out @ wo.T).astype(np.float32)


def kernel(x, mask, wq, wk, wv, wo):
    import ml_dtypes

    bf16 = ml_dtypes.bfloat16

    x = np.ascontiguousarray(np.asarray(x, dtype=np.float32))
    mask = np.asarray(mask, dtype=np.float32)
    wq = np.ascontiguousarray(np.asarray(wq, dtype=np.float32))
    wk = np.ascontiguousarray(np.asarray(wk, dtype=np.float32))
    wv = np.ascontiguousarray(np.asarray(wv, dtype=np.float32))
    wo = np.ascontiguousarray(np.asarray(wo, dtype=np.float32))

    if not _is_causal_neg_mask(mask):
        return _reference_fallback(x, mask, wq, wk, wv, wo)

    from concourse.bass_utils import run_bass_kernel_spmd

    nc = _get_program()

    xT = x.reshape(SG, DIM).T  # [DIM, SG]
    # xS[cg, p, kc, s'] = xT[kc*128+p, cg*AC+s'] (contiguous per chunk)
    xS = xT.reshape(KC, 128, SG // AC, AC).transpose(2, 1, 0, 3).astype(
        bf16, order="C"
    )
    # m01big[k, c] = 1.0 iff (c - 384) >= k; partial blocks slice [384:384+N)
    kk = np.arange(128)[:, None]
    cc = np.arange(1024)[None, :]
    m01x = ((cc - 384) >= kk).astype(bf16)
    onesA = np.ones((128, 1), dtype=np.float32)
    onesB = np.ones((1, 128), dtype=np.float32)

    in_maps = []
    for c in range(NCORES):
        hs = slice(c * DPC, (c + 1) * DPC)
        in_maps.append(
            {
                "xS": xS,
                "wqT": wq[hs, :].T.astype(bf16, order="C"),
                "wkT": wk[hs, :].T.astype(bf16, order="C"),
                "wvT": wv[hs, :].T.astype(bf16, order="C"),
                "woT": wo[:, hs].T.astype(bf16, order="C"),
                "m01x": m01x,
                "onesA": onesA,
                "onesB": onesB,
            }
        )

    global LAST_RESULT
    for attempt in range(3):
        res = run_bass_kernel_spmd(nc, in_maps, list(range(NCORES)))
        LAST_RESULT = res
        acc = np.asarray(res.results[0]["outp"]).astype(np.float32)
        for c in range(1, NCORES):
            acc += np.asarray(res.results[c]["outp"]).astype(np.float32)
        # guard against rare transient device glitches (non-finite output)
        if np.isfinite(acc).all():
            break
    # outp is out.T: [m, s_glob] -> [B, S, DIM]
    return np.ascontiguousarray(acc.T).reshape(B, S, DIM)


if __name__ == "__main__":
    rng = np.random.default_rng(0)
    x = rng.standard_normal((B, S, DIM), dtype=np.float32)
    neg = np.float32(-1e9)
    maskm = np.triu(np.full((S, S), neg, dtype=np.float32), k=1)[None, None]
    ws = [rng.standard_normal((DIM, DIM), dtype=np.float32) * 0.02 for _ in range(4)]
    out = kernel(x, maskm, *ws)
    print(out.shape, out.dtype)


# revision 12
# speedup vs baseline: 1.2046x; 1.1197x over previous
"""TRN2 Bass kernel for nn_Attention (B=2, S=2048, DIM=2048, 16 heads).

Sharding: tensor-parallel over heads — 8 cores x 2 heads each.
Each core computes q/k/v projections for its 2 heads over both batches,
causal attention, and a partial output projection (row-parallel wo).
Host sums the 8 partial outputs.

Layouts (per core):
  xS   [8, 128, 16, 512]   = x.T chunked contiguous per s-chunk (replicated)
  wqT  [2048(k), 256(dq)]  = wq[head rows].T                  (sharded)
  wkT, wvT likewise; woT [256(dc), 2048(m)] = wo[:, head cols].T
  outp [2048(m), 4096(s)]  partial of out.T (bf16, summed on host)

All matmul operands are bf16 (PSUM accumulation stays fp32), except the
softmax-denominator path which runs in float32r. The denominator is
accumulated per key-block into exacc tiles (split across Vector and GpSimd
engines), then folded across partitions with a single ones-matmul per
(batch, chunk, head) instead of one per key-block.
"""

import sys

sys.path.insert(0, "/opt/trn_rl_repo")

import numpy as np

DIM = 2048
HEADS = 16
HD = 128
B = 2
S = 2048
SG = B * S  # 4096 global sequence (batch-major)
NCORES = 8
HPC = HEADS // NCORES  # 2 heads per core
DPC = HPC * HD  # 256 dims per core
KC = DIM // 128  # 16 contraction chunks
AC = 512  # chunk width (projection and attention)
NAC = S // AC  # 4 chunks per batch
ISQ = 1.0 / np.sqrt(np.float32(HD))

_prog_cache = {}


def _build_program():
    import concourse.bass as bass
    from concourse import bacc
    import concourse.mybir as mybir
    import concourse.tile as tile

    # Route Exp AND Ln to the one table set containing both, so the ACT
    # table is loaded once instead of thrashing between per-function sets
    # (~1.3us per reload, 2 reloads per softmax-normalize otherwise).
    if not getattr(bacc, "_act_tables_patched", False):
        _orig_get_tables = bacc.get_activation_tables
        _E = mybir.ActivationFunctionType.Exp
        _L = mybir.ActivationFunctionType.Ln

        def _patched_get_tables(arch):
            tabs = dict(_orig_get_tables(arch))
            both = {
                n for n, fns in tabs.items() if _E in fns and _L in fns
            }
            if both:
                keep = sorted(both)[0]
                tabs = {
                    n: (fns if n == keep else fns - {_E, _L})
                    for n, fns in tabs.items()
                }
            return tabs

        bacc.get_activation_tables = _patched_get_tables
        bacc._act_tables_patched = True

    f32 = mybir.dt.float32
    fr = mybir.dt.float32r
    bf = mybir.dt.bfloat16
    EXP = mybir.ActivationFunctionType.Exp
    LOG = mybir.ActivationFunctionType.Ln

    nc = bacc.Bacc()

    xS = nc.dram_tensor("xS", [SG // AC, 128, KC, AC], bf, kind="ExternalInput")
    wqT = nc.dram_tensor("wqT", [DIM, DPC], bf, kind="ExternalInput")
    wkT = nc.dram_tensor("wkT", [DIM, DPC], bf, kind="ExternalInput")
    wvT = nc.dram_tensor("wvT", [DIM, DPC], bf, kind="ExternalInput")
    woT = nc.dram_tensor("woT", [DPC, DIM], bf, kind="ExternalInput")
    m01x = nc.dram_tensor("m01x", [128, 1024], bf, kind="ExternalInput")
    onesA = nc.dram_tensor("onesA", [128, 1], fr, kind="ExternalInput")
    onesB = nc.dram_tensor("onesB", [1, 128], fr, kind="ExternalInput")
    outp = nc.dram_tensor("outp", [DIM, SG], bf, kind="ExternalOutput")

    with tile.TileContext(nc) as tc:
        with (
            tc.tile_pool(name="wpool", bufs=1) as wpool,
            tc.tile_pool(name="xpool", bufs=3) as xpool,
            tc.tile_pool(name="kv", bufs=1) as kvpool,
            tc.tile_pool(name="work", bufs=2) as work,
            tc.tile_pool(name="expool", bufs=3) as expool,
            tc.tile_pool(name="ps", bufs=1, space="PSUM") as ps,
        ):
            # --- resident constants / weights ---
            wqr = wpool.tile([128, KC, DPC], bf, tag="wqr")
            wkr = wpool.tile([128, KC, DPC], bf, tag="wkr")
            wvr = wpool.tile([128, KC, DPC], bf, tag="wvr")
            wor = wpool.tile([128, HPC, DIM], bf, tag="wor")
            m01 = wpool.tile([128, 1024], bf, tag="m01")
            onA = wpool.tile([128, 1], fr, tag="onA")
            onB = wpool.tile([1, 128], fr, tag="onB")

            def emit_w_dmas(wtile, wdram):
                for kc in range(KC):
                    ksl = slice(kc * 128, (kc + 1) * 128)
                    nc.sync.dma_start(wtile[:, kc, :], wdram[ksl, :])

            def emit_late_dmas():
                for dc in range(HPC):
                    nc.sync.dma_start(
                        wor[:, dc, :], woT[dc * 128 : (dc + 1) * 128, :]
                    )
                nc.sync.dma_start(onA[:], onesA[:])
                nc.sync.dma_start(onB[:], onesB[:])

            # resident per-core activations
            kTr = kvpool.tile([128, B * HPC, S], bf, tag="kTr")  # [d, bh, s]
            vr = kvpool.tile([128, B * (S // 128), DPC], bf, tag="vr")  # [s%, blk, d]

            def proj_units(b, j, qTc):
                xa = xpool.tile([128, KC, AC], bf, tag="xa", name=f"xa_{b}_{j}")
                cg = b * NAC + j

                def dma_unit(xa=xa, cg=cg):
                    for qt in range(8):
                        nc.sync.dma_start(
                            xa[:, qt * 2 : (qt + 1) * 2, :],
                            xS[cg, :, qt * 2 : (qt + 1) * 2, :],
                        )

                def q_unit(h, xa=xa):
                    dsl = slice(h * 128, (h + 1) * 128)
                    pq = ps.tile([128, AC], f32, tag="pq", bufs=2)
                    for kc in range(KC):
                        nc.tensor.matmul(
                            pq[:], wqr[:, kc, dsl], xa[:, kc, :],
                            start=(kc == 0), stop=(kc == KC - 1),
                        )
                    nc.vector.tensor_copy(qTc[:, h, :], pq[:])

                def k_unit(h, xa=xa):
                    dsl = slice(h * 128, (h + 1) * 128)
                    pk = ps.tile([128, AC], f32, tag="pq", bufs=2)
                    for kc in range(KC):
                        nc.tensor.matmul(
                            pk[:], wkr[:, kc, dsl], xa[:, kc, :],
                            start=(kc == 0), stop=(kc == KC - 1),
                        )
                    nc.vector.tensor_copy(
                        kTr[:, b * HPC + h, j * AC : (j + 1) * AC], pk[:]
                    )

                def v_unit(sb, xa=xa):
                    pv = ps.tile([128, DPC], f32, tag="pq", bufs=2)
                    for kc in range(KC):
                        nc.tensor.matmul(
                            pv[:], xa[:, kc, sb * 128 : (sb + 1) * 128],
                            wvr[:, kc, :],
                            start=(kc == 0), stop=(kc == KC - 1),
                        )
                    vblk = b * (S // 128) + j * (AC // 128) + sb
                    nc.vector.tensor_copy(vr[:, vblk, :], pv[:])

                units = [
                    lambda: q_unit(0), lambda: k_unit(0),
                    lambda: v_unit(0), lambda: v_unit(1),
                    lambda: q_unit(1), lambda: k_unit(1),
                    lambda: v_unit(2), lambda: v_unit(3),
                ]
                return [dma_unit] + units

            def att_units(b, j, qTc, uS):
                units = []
                for h in range(HPC):
                    bh = b * HPC + h
                    nblocks = (j + 1) * (AC // 128)
                    nfull = j * (AC // 128)
                    box = {}

                    def head_start(box=box, h=h):
                        box["U"] = ps.tile([128, AC], f32, tag="u", bufs=2,
                                           name=f"U_{b}_{j}_{h}")
                        # four denominator accumulators (round-robin over key
                        # blocks, alternating DVE / GpSimd) — keeps each
                        # serial add chain short so the normalize step never
                        # stalls the endgame
                        box["accs"] = [
                            work.tile([128, AC], fr, tag=f"e{r}", bufs=2,
                                      name=f"e{r}_{b}_{j}_{h}")
                            for r in range(4)
                        ]
                        box["l"] = [0, 0, 0, 0]

                    for i in range(nblocks):
                        def block_unit(i=i, h=h, bh=bh, box=box,
                                       nblocks=nblocks, nfull=nfull):
                            if i == 0:
                                head_start(box, h)
                            U = box["U"]
                            loc = max(0, 128 * i - AC * j)
                            sc = ps.tile([128, AC], f32, tag="sc", bufs=2)
                            ex = expool.tile([128, AC], bf, tag="ex", bufs=5)
                            nc.tensor.matmul(
                                sc[:, loc:AC],
                                kTr[:, bh, i * 128 : (i + 1) * 128],
                                qTc[:, h, loc:AC],
                                start=True, stop=True,
                            )
                            if i < nfull:
                                nc.scalar.activation(ex[:], sc[:], EXP, scale=ISQ)
                            else:
                                ds = expool.tile([128, AC], bf, tag="ds", bufs=2)
                                nc.scalar.activation(
                                    ds[:, loc:AC], sc[:, loc:AC], EXP, scale=ISQ
                                )
                                nc.vector.tensor_mul(
                                    ex[:, loc:AC], ds[:, loc:AC],
                                    m01[:, 384 : 384 + AC - loc],
                                )
                            r = i % 4
                            acc = box["accs"][r]
                            eng = nc.vector if r % 2 == 0 else nc.gpsimd
                            if i < 4:
                                # first block per accumulator may be diagonal
                                # (only [loc:AC) written) — record the offset
                                # so the se fold only reads the written range
                                box["l"][r] = loc
                                eng.tensor_copy(acc[:, loc:AC], ex[:, loc:AC])
                            else:
                                eng.tensor_add(
                                    acc[:, loc:AC], acc[:, loc:AC], ex[:, loc:AC]
                                )
                            vblk = b * (S // 128) + i
                            nc.tensor.matmul(
                                U[:, loc:AC],
                                vr[:, vblk, h * 128 : (h + 1) * 128],
                                ex[:, loc:AC],
                                start=(i == 0), stop=(i == nblocks - 1),
                            )

                        units.append(block_unit)

                    def red_unit(h=h, box=box):
                        se = ps.tile([1, AC], f32, tag="se", bufs=1,
                                     name=f"se_{b}_{j}_{h}")
                        for r in range(4):
                            lr = box["l"][r]
                            nc.tensor.matmul(
                                se[:, lr:AC], onA[:], box["accs"][r][:, lr:AC],
                                start=(r == 0), stop=(r == 3),
                            )
                        lnz = work.tile([1, AC], fr, tag="lnz",
                                        name=f"lnz_{b}_{j}_{h}")
                        nc.scalar.activation(lnz[:], se[:], LOG)
                        box["lnz"] = lnz

                    def fin_unit(h=h, box=box):
                        bc = ps.tile([128, AC], f32, tag="sc", bufs=2)
                        nc.tensor.matmul(
                            bc[:], onB[:], box["lnz"][:], start=True, stop=True
                        )
                        rb = work.tile([128, AC], f32, tag="rb")
                        nc.scalar.activation(rb[:], bc[:], EXP, scale=-1.0)
                        nc.vector.tensor_mul(uS[:, h, :], box["U"][:], rb[:])

                    units.append(red_unit)
                    units.append(fin_unit)
                return units

            def out_units(b, j, uS, tags=("po",)):
                units = []
                sg0 = b * S + j * AC
                for mb in range(DIM // 128):
                    def o_unit(mb=mb):
                        tg = tags[mb % len(tags)]
                        po = ps.tile(
                            [128, AC], f32, tag=tg, bufs=(1 if tg == "po" else 2)
                        )
                        for dc in range(HPC):
                            nc.tensor.matmul(
                                po[:],
                                wor[:, dc, mb * 128 : (mb + 1) * 128],
                                uS[:, dc, :],
                                start=(dc == 0), stop=(dc == HPC - 1),
                            )
                        ob = work.tile([128, AC], bf, tag="ob")
                        if mb % 3 == 2:
                            nc.scalar.copy(ob[:], po[:])
                        else:
                            nc.vector.tensor_copy(ob[:], po[:])
                        nc.sync.dma_start(
                            outp[mb * 128 : (mb + 1) * 128, sg0 : sg0 + AC], ob[:]
                        )

                    units.append(o_unit)
                return units

            def merge_emit(a_units, b_units):
                na, nb = len(a_units), len(b_units)
                ia = ib = 0
                while ia < na or ib < nb:
                    fa = ia / na if na else 2.0
                    fb = ib / nb if nb else 2.0
                    if fa <= fb:
                        a_units[ia]()
                        ia += 1
                    else:
                        b_units[ib]()
                        ib += 1

            # software pipeline: att(c) interleaved with proj(c+1) + out(c-1)
            # batches interleaved so the final att chunk still has the other
            # batch's out-projection as PE fill work
            chunks = [(b, j) for j in range(NAC) for b in range(B)]
            qTcs = {}
            uSs = {}
            qTcs[chunks[0]] = work.tile([128, HPC, AC], bf, tag="qTc", name="qTc0")
            u0 = proj_units(*chunks[0], qTcs[chunks[0]])
            # startup order: chunk-0 x DMAs, then weights in first-use order
            # (wq for the q matmuls, wk, wv+mask), then chunk-0 compute; the
            # wo/ones DMAs are emitted only after the first chunk's work.
            u0[0]()
            emit_w_dmas(wqr, wqT)
            emit_w_dmas(wkr, wkT)
            emit_w_dmas(wvr, wvT)
            nc.sync.dma_start(m01[:], m01x[:])
            for u in u0[1:]:
                u()
            emit_late_dmas()
            for idx, (b, j) in enumerate(chunks):
                fill = []
                if idx + 1 < len(chunks):
                    nb_, nj_ = chunks[idx + 1]
                    qTcs[(nb_, nj_)] = work.tile(
                        [128, HPC, AC], bf, tag="qTc", name=f"qTc_{nb_}_{nj_}"
                    )
                    fill += proj_units(nb_, nj_, qTcs[(nb_, nj_)])
                if idx > 0:
                    fill += out_units(*chunks[idx - 1], uSs.pop(chunks[idx - 1]))
                uS = work.tile([128, HPC, AC], bf, tag="uS", name=f"uS_{b}_{j}")
                uSs[(b, j)] = uS
                merge_emit(att_units(b, j, qTcs.pop((b, j)), uS), fill)
            for u in out_units(*chunks[-1], uSs.pop(chunks[-1]), tags=("po", "u")):
                u()

    nc.finalize()
    return nc


def _get_program():
    key = "prog"
    if key not in _prog_cache:
        _prog_cache[key] = _build_program()
    return _prog_cache[key]


def _is_causal_neg_mask(mask):
    m = mask.reshape(S, S)
    tri = np.triu(np.ones((S, S), dtype=bool), k=1)
    return (
        np.all(m[~tri] == 0.0)
        and np.all(m[tri] <= -1e8)
        and np.all(np.isfinite(m) | tri)
    )


def _reference_fallback(x, mask, wq, wk, wv, wo):
    xf = x.astype(np.float32)
    q = (xf @ wq.T).reshape(B, S, HEADS, HD).transpose(0, 2, 1, 3)
    k = (xf @ wk.T).reshape(B, S, HEADS, HD).transpose(0, 2, 1, 3)
    v = (xf @ wv.T).reshape(B, S, HEADS, HD).transpose(0, 2, 1, 3)
    scores = np.matmul(q, k.transpose(0, 1, 3, 2)) / np.sqrt(np.float32(HD))
    scores = scores + mask
    scores = scores - scores.max(axis=-1, keepdims=True)
    e = np.exp(scores)
    probs = e / e.sum(axis=-1, keepdims=True)
    out = np.matmul(probs, v)
    out = out.transpose(0, 2, 1, 3).reshape(B, S, HEADS * HD)
    return (out @ wo.T).astype(np.float32)


def kernel(x, mask, wq, wk, wv, wo):
    import ml_dtypes

    bf16 = ml_dtypes.bfloat16

    x = np.ascontiguousarray(np.asarray(x, dtype=np.float32))
    mask = np.asarray(mask, dtype=np.float32)
    wq = np.ascontiguousarray(np.asarray(wq, dtype=np.float32))
    wk = np.ascontiguousarray(np.asarray(wk, dtype=np.float32))
    wv = np.ascontiguousarray(np.asarray(wv, dtype=np.float32))
    wo = np.ascontiguousarray(np.asarray(wo, dtype=np.float32))

    if not _is_causal_neg_mask(mask):
        return _reference_fallback(x, mask, wq, wk, wv, wo)

    from concourse.bass_utils import run_bass_kernel_spmd

    nc = _get_program()

    xT = x.reshape(SG, DIM).T  # [DIM, SG]
    # xS[cg, p, kc, s'] = xT[kc*128+p, cg*AC+s'] (contiguous per chunk)
    xS = xT.reshape(KC, 128, SG // AC, AC).transpose(2, 1, 0, 3).astype(
        bf16, order="C"
    )
    # m01big[k, c] = 1.0 iff (c - 384) >= k; partial blocks slice [384:384+N)
    kk = np.arange(128)[:, None]
    cc = np.arange(1024)[None, :]
    m01x = ((cc - 384) >= kk).astype(bf16)
    onesA = np.ones((128, 1), dtype=np.float32)
    onesB = np.ones((1, 128), dtype=np.float32)

    in_maps = []
    for c in range(NCORES):
        hs = slice(c * DPC, (c + 1) * DPC)
        in_maps.append(
            {
                "xS": xS,
                "wqT": wq[hs, :].T.astype(bf16, order="C"),
                "wkT": wk[hs, :].T.astype(bf16, order="C"),
                "wvT": wv[hs, :].T.astype(bf16, order="C"),
                "woT": wo[:, hs].T.astype(bf16, order="C"),
                "m01x": m01x,
                "onesA": onesA,
                "onesB": onesB,
            }
        )

    global LAST_RESULT
    for attempt in range(3):
        res = run_bass_kernel_spmd(nc, in_maps, list(range(NCORES)))
        LAST_RESULT = res
        acc = np.asarray(res.results[0]["outp"]).astype(np.float32)
        for c in range(1, NCORES):
            acc += np.asarray(res.results[c]["outp"]).astype(np.float32)
        # guard against rare transient device glitches (non-finite output)
        if np.isfinite(acc).all():
            break
    # outp is out.T: [m, s_glob] -> [B, S, DIM]
    return np.ascontiguousarray(acc.T).reshape(B, S, DIM)


if __name__ == "__main__":
    rng = np.random.default_rng(0)
    x = rng.standard_normal((B, S, DIM), dtype=np.float32)
    neg = np.float32(-1e9)
    maskm = np.triu(np.full((S, S), neg, dtype=np.float32), k=1)[None, None]
    ws = [rng.standard_normal((DIM, DIM), dtype=np.float32) * 0.02 for _ in range(4)]
    out = kernel(x, maskm, *ws)
    print(out.shape, out.dtype)
